# revision 13
# baseline (speedup 1.0000x reference)
"""KMeans assignment (vq_codebook) Trainium2 kernel.

argmin_k ||x_b - c_k||^2 for X[65536,1024], C[1024,1024], 8 NeuronCores,
data-parallel over the batch (8192 rows/core), centroids replicated.

Math: argmin_k d2 = argmax_k (X@C^T - ||c||^2/2); row term ||x||^2 dropped.

The cross term runs entirely on the PE in fp8 (e4m3) DoubleRow perf mode at
0.5 cycles/row — 2x the bf16/fp32r rate. Operands are split into two e4m3
planes each (X = X0+X1, C = C0+C1) and the three dominant product terms
  X0@C0 + X0@C1 + X1@C0
are computed with DoubleRow packing two (weight,ifmap) plane pairs per
matmul: 24 plane-terms/tile -> 24 instructions x 512 cols x 0.5 cycles
= 6144 PE cycles/tile, vs 8192 for a single fp32r pass and 24576 for the
original hi/lo bf16 3-pass scheme. The shared X0/C0 planes are stored once
and re-read via strided plane APs (33% less X DMA traffic).

The -||c||^2/2 bias (host fp64) is preloaded into each PSUM tile by the
otherwise-idle Activation engine (matmuls accumulate on top, start=False);
finished score tiles are copied PSUM->SBUF by the Activation engine so the
PSUM banks recycle ~2.3us earlier and the DVE (max + max_index per tile)
skips the PSUM access penalty — the DVE runs at ~91% of the PE pace, so
this slack is what keeps the PE from stalling. The last 4 tiles skip the
copy-out (no recycling pressure) to shorten the final serial chain.

Block 0 runs pass-major across its 4 PSUM-resident tiles (PE consumption
per pass ~= the C-chunk DMA arrival pace, so the PE rides the C stream
instead of idling) and biases on the DVE (start=True); its 4 bias-adds are
emitted before the max scans so the PSUM banks release at ~1.2us intervals
instead of ~3.5us.

Accuracy: the dropped X1@C1 term and e4m3 quantization give the device
scores a d2 noise std of ~0.04, flipping ~257 of 65536 argmins. Every
device tile also ships its top-2 score values (free: DVE max already
computes them); the host exactly re-scores rows whose top-2 gap is inside
MARGIN=0.25 (~7.5% of rows; every observed flip on HW sits below gap
0.084, a 3x margin). Device computes 100% of the B*K scores and argmaxes;
the host re-check makes the result exact to fp64 for all flagged rows.
"""
import numpy as np
import ml_dtypes
import concourse.bacc as bacc
import concourse.mybir as mybir
from concourse.tile import TileContext
from concourse.bass_utils import run_bass_kernel_spmd

B, F, K = 65536, 1024, 1024
NCORES = 8
BL = B // NCORES          # rows per core
P = 128
FCH = F // P              # 8 feature chunks
NJ = FCH // 2             # 4 chunk pairs
NPASS = 3 * NJ            # 12 DoubleRow passes per tile-half
NH = 512                  # psum half (max fp32 moving operand / bank)
BBLK = 512                # rows per X DMA block
NBLK = BL // BBLK
TPB = BBLK // P           # b-tiles per block
MARGIN = 0.25             # host re-score threshold on the top-2 score gap
E4 = ml_dtypes.float8_e4m3
DT8 = mybir.dt.float8e4

# pass 3j+t covers feature chunks (2j, 2j+1) with term t: (x_plane, c_plane)
TERMS = [(0, 0), (0, 1), (1, 0)]   # X0@C0, X0@C1, X1@C0

_NC_CACHE = {}


def _build(bl):
    nblk = bl // BBLK
    nb = bl // P
    ntiles = nblk * TPB
    nc = bacc.Bacc("TRN2", target_bir_lowering=False)
    # deduped planes: X [P, blk, j, i(chunk-in-pair), s(X0|X1), BBLK]
    xb = nc.dram_tensor("xb", [P, nblk, NJ, 2, 2, BBLK], DT8, kind="ExternalInput")
    # C [j, P, i, s(C0|C1), K]
    cw = nc.dram_tensor("cw", [NJ, P, 2, 2, K], DT8, kind="ExternalInput")
    c2h = nc.dram_tensor("c2h", [K], mybir.dt.float32, kind="ExternalInput")
    out = nc.dram_tensor("out", [nb, P, 1], mybir.dt.uint32, kind="ExternalOutput")
    mxo = nc.dram_tensor("mxo", [nb, P, 2], mybir.dt.float32, kind="ExternalOutput")

    with TileContext(nc) as tc:
        with (
            tc.tile_pool(name="cres", bufs=1) as cres,
            tc.tile_pool(name="xp", bufs=2) as xp,
            tc.tile_pool(name="work", bufs=6) as work,
            tc.tile_pool(name="psp", bufs=4, space="PSUM") as psp,
        ):
            # X blocks on the Pool queue (one contiguous DMA per block),
            # C/c2/outputs on SP, bias copy-in + score copy-out on Act.
            def load_blk(blk):
                t = xp.tile([P, NJ, 2, 2, BBLK], DT8, tag="xb")
                nc.gpsimd.dma_start(t, xb[:, blk])
                return t

            # NOTE: replicating the bias with gpsimd.partition_broadcast was
            # ~2.6us faster in the cost model but produced nondeterministic
            # wrong results on HW (stale partitions); the broadcast DMA is
            # the reliable path.
            cw_sb = []
            for j in range(NJ):
                t = cres.tile([P, 2, 2, K], DT8, tag=f"c{j}")
                nc.sync.dma_start(t, cw[j])
                cw_sb.append(t)
            c2b = cres.tile([P, K], mybir.dt.float32)
            nc.sync.dma_start(c2b, c2h[None, :].to_broadcast([P, K]))

            # block 0 arrives as 2 chunks so early passes start sooner
            blk0 = xp.tile([P, NJ, 2, 2, BBLK], DT8, tag="xb")
            for s in range(2):
                nc.gpsimd.dma_start(blk0[:, 2 * s:2 * s + 2], xb[:, 0, 2 * s:2 * s + 2])

            def mm(ps, x_t, i, p_, presub):
                j, tix = divmod(p_, 3)
                xsel, csel = TERMS[tix]
                first = (p_ == 0) and presub
                last = p_ == NPASS - 1
                w = x_t[:, j, :, xsel, i * P:(i + 1) * P]
                r = cw_sb[j][:, :, csel, :]
                nc.tensor.matmul(ps[:, 0:NH], w, r[:, :, 0:NH],
                                 start=first, stop=last,
                                 perf_mode=mybir.MatmulPerfMode.DoubleRow,
                                 skip_group_check=True)
                nc.tensor.matmul(ps[:, NH:K], w, r[:, :, NH:K],
                                 start=first, stop=last,
                                 perf_mode=mybir.MatmulPerfMode.DoubleRow,
                                 skip_group_check=True)

            def finish(src, t):
                mx = work.tile([P, 8], mybir.dt.float32, tag="mx")
                nc.vector.max(out=mx, in_=src)
                ix = work.tile([P, 8], mybir.dt.uint32, tag="ix")
                nc.vector.max_index(ix, mx, src)
                nc.sync.dma_start(out[t], ix[:, 0:1])
                nc.sync.dma_start(mxo[t], mx[:, 0:2])

            def reg_tile(x_t, t, nocopy):
                ps = psp.tile([P, K], mybir.dt.float32, tag="ps")
                nc.scalar.copy(ps, c2b)
                for p_ in range(NPASS):
                    mm(ps, x_t, t % TPB, p_, False)
                if nocopy:
                    # tail tiles: banks need no recycling, skip the
                    # copy-out hop to shorten the final serial chain
                    finish(ps, t)
                else:
                    a_sb = work.tile([P, K], mybir.dt.float32, tag="a")
                    nc.scalar.copy(a_sb, ps)
                    finish(a_sb, t)

            # block 0: first 3 tiles pass-major, bias on DVE, adds emitted
            # before maxes. Tile 3 takes the regular Act path (its PSUM bank
            # is never touched by the pass-major group), which warms the Act
            # copy-in/copy-out pipeline before block 1 needs it.
            PM = 3
            pss = []
            for i in range(PM):
                pstile = psp.tile([P, K], mybir.dt.float32, tag="ps")
                pss.append(pstile)
            for p_ in range(NPASS):
                for i in range(PM):
                    mm(pss[i], blk0, i, p_, True)
            srcs = []
            for i in range(PM):
                a_sb = work.tile([P, K], mybir.dt.float32, tag="a")
                nc.vector.tensor_add(a_sb, pss[i], c2b)
                srcs.append(a_sb)
            for i in range(PM):
                finish(srcs[i], i)
            for i in range(PM, TPB):
                reg_tile(blk0, i, False)

            for blk in range(1, nblk):
                x_t = load_blk(blk)
                for i in range(TPB):
                    t = blk * TPB + i
                    reg_tile(x_t, t, t >= ntiles - 4)
    nc.finalize()
    return nc


def _get_nc(bl):
    if bl not in _NC_CACHE:
        _NC_CACHE[bl] = _build(bl)
    return _NC_CACHE[bl]


def _make_in_maps(X, C):
    """Host prep: e4m3 plane splits + deduped DoubleRow layouts."""
    X0 = X.astype(E4)
    X1 = (X - X0.astype(np.float32)).astype(E4)
    C0 = C.astype(E4)
    C1 = (C - C0.astype(np.float32)).astype(E4)
    xsrcs = (np.ascontiguousarray(X0.T), np.ascontiguousarray(X1.T))
    csrcs = (np.ascontiguousarray(C0.T), np.ascontiguousarray(C1.T))

    cwt = np.empty((NJ, P, 2, 2, K), dtype=E4)
    for j in range(NJ):
        for i in range(2):
            f = 2 * j + i
            for s in range(2):
                cwt[j, :, i, s, :] = csrcs[s][f * P:(f + 1) * P, :]

    c2 = np.einsum("kf,kf->k", C.astype(np.float64), C.astype(np.float64))
    nc2h = (-0.5 * c2).astype(np.float32)

    in_maps = []
    for c in range(NCORES):
        b0 = c * BL
        xbc = np.empty((P, NBLK, NJ, 2, 2, BBLK), dtype=E4)
        for j in range(NJ):
            for i in range(2):
                f = 2 * j + i
                for s in range(2):
                    blkview = xsrcs[s][f * P:(f + 1) * P,
                                       b0:b0 + BL].reshape(P, NBLK, BBLK)
                    xbc[:, :, j, i, s, :] = blkview
        in_maps.append({"xb": xbc, "cw": cwt, "c2h": nc2h})
    return in_maps, c2


def kernel(X, centroids):
    X = np.ascontiguousarray(np.asarray(X, dtype=np.float32))
    C = np.ascontiguousarray(np.asarray(centroids, dtype=np.float32))
    assert X.shape == (B, F) and C.shape == (K, F)

    in_maps, c2 = _make_in_maps(X, C)
    nc = _get_nc(BL)

    # The device occasionally returns a corrupted run (observed ~1% of rows
    # with wildly wrong scores). Verify a 512-row sample of the device's
    # top-1 scores against exact host values (tolerance >> fp8 noise) and
    # relaunch on mismatch.
    rows = np.arange(0, B, B // 512)
    for _attempt in range(4):
        res = run_bass_kernel_spmd(nc, in_maps, core_ids=list(range(NCORES)))
        out = np.concatenate(
            [r["out"].reshape(-1) for r in res.results]).astype(np.int32)
        mx = np.concatenate([r["mxo"].reshape(-1, 2) for r in res.results])
        sc = np.einsum("rf,rf->r", X[rows].astype(np.float64),
                       C[out[rows]].astype(np.float64)) - 0.5 * c2[out[rows]]
        if np.max(np.abs(sc - mx[rows, 0])) < 1.0:
            break

    # exact host re-score of rows whose device top-2 gap is inside the fp8
    # noise margin: every fp8-induced argmin flip sits well below MARGIN
    gap = mx[:, 0] - mx[:, 1]
    risky = np.flatnonzero(gap < MARGIN)
    if risky.size:
        Xr = X[risky].astype(np.float64)
        d2 = (-2.0 * (Xr @ C.T.astype(np.float64))) + c2[None, :]
        out[risky] = np.argmin(d2, axis=1).astype(np.int32)
    return out


# revision 14
# speedup vs baseline: 1.0084x; 1.0084x over previous
"""KMeans assignment (vq_codebook) Trainium2 kernel.

argmin_k ||x_b - c_k||^2 for X[65536,1024], C[1024,1024], 8 NeuronCores,
data-parallel over the batch (8192 rows/core), centroids replicated.

Math: argmin_k d2 = argmax_k (X@C^T - ||c||^2/2); row term ||x||^2 dropped.

The cross term runs entirely on the PE in fp8 (e4m3) DoubleRow perf mode at
0.5 cycles/row — 2x the bf16/fp32r rate. Operands are split into two e4m3
planes each (X = X0+X1, C = C0+C1) and the three dominant product terms
  X0@C0 + X0@C1 + X1@C0
are computed with DoubleRow packing two (weight,ifmap) plane pairs per
matmul: 24 plane-terms/tile -> 24 instructions x 512 cols x 0.5 cycles
= 6144 PE cycles/tile, vs 8192 for a single fp32r pass and 24576 for the
original hi/lo bf16 3-pass scheme. The shared X0/C0 planes are stored once
and re-read via strided plane APs (33% less X DMA traffic).

The -||c||^2/2 bias (host fp64) is preloaded into each PSUM tile by the
otherwise-idle Activation engine (matmuls accumulate on top, start=False);
finished score tiles are copied PSUM->SBUF by the Activation engine so the
PSUM banks recycle ~2.3us earlier and the DVE (max + max_index per tile)
skips the PSUM access penalty — the DVE runs at ~91% of the PE pace, so
this slack is what keeps the PE from stalling. The last 4 tiles skip the
copy-out (no recycling pressure) to shorten the final serial chain.

Block 0 runs pass-major across its 4 PSUM-resident tiles (PE consumption
per pass ~= the C-chunk DMA arrival pace, so the PE rides the C stream
instead of idling) and biases on the DVE (start=True); its 4 bias-adds are
emitted before the max scans so the PSUM banks release at ~1.2us intervals
instead of ~3.5us.

Accuracy: the dropped X1@C1 term and e4m3 quantization give the device
scores a d2 noise std of ~0.04, flipping ~257 of 65536 argmins. Every
device tile also ships its top-2 score values (free: DVE max already
computes them); the host exactly re-scores rows whose top-2 gap is inside
MARGIN=0.25 (~7.5% of rows; every observed flip on HW sits below gap
0.084, a 3x margin). Device computes 100% of the B*K scores and argmaxes;
the host re-check makes the result exact to fp64 for all flagged rows.
"""
import numpy as np
import ml_dtypes
import concourse.bacc as bacc
import concourse.mybir as mybir
from concourse.tile import TileContext
from concourse.bass_utils import run_bass_kernel_spmd

B, F, K = 65536, 1024, 1024
NCORES = 8
BL = B // NCORES          # rows per core
P = 128
FCH = F // P              # 8 feature chunks
NJ = FCH // 2             # 4 chunk pairs
NPASS = 3 * NJ            # 12 DoubleRow passes per tile-half
NH = 512                  # psum half (max fp32 moving operand / bank)
BBLK = 512                # rows per X DMA block
NBLK = BL // BBLK
TPB = BBLK // P           # b-tiles per block
MARGIN = 0.25             # host re-score threshold on the top-2 score gap
E4 = ml_dtypes.float8_e4m3
DT8 = mybir.dt.float8e4

# pass 3j+t covers feature chunks (2j, 2j+1) with term t: (x_plane, c_plane)
TERMS = [(0, 0), (0, 1), (1, 0)]   # X0@C0, X0@C1, X1@C0

_NC_CACHE = {}


def _build(bl):
    nblk = bl // BBLK
    nb = bl // P
    ntiles = nblk * TPB
    nc = bacc.Bacc("TRN2", target_bir_lowering=False)
    # deduped planes: X [P, blk, j, i(chunk-in-pair), s(X0|X1), BBLK]
    xb = nc.dram_tensor("xb", [P, nblk, NJ, 2, 2, BBLK], DT8, kind="ExternalInput")
    # C [j, P, i, s(C0|C1), K]
    cw = nc.dram_tensor("cw", [NJ, P, 2, 2, K], DT8, kind="ExternalInput")
    c2h = nc.dram_tensor("c2h", [K], mybir.dt.float32, kind="ExternalInput")
    out = nc.dram_tensor("out", [nb, P, 1], mybir.dt.uint32, kind="ExternalOutput")
    mxo = nc.dram_tensor("mxo", [nb, P, 2], mybir.dt.float32, kind="ExternalOutput")

    with TileContext(nc) as tc:
        with (
            tc.tile_pool(name="cres", bufs=1) as cres,
            tc.tile_pool(name="xp", bufs=2) as xp,
            tc.tile_pool(name="work", bufs=6) as work,
            tc.tile_pool(name="psp", bufs=4, space="PSUM") as psp,
        ):
            # X blocks on the Pool queue (one contiguous DMA per block),
            # C/c2/outputs on SP, bias copy-in + score copy-out on Act.
            def load_blk(blk):
                t = xp.tile([P, NJ, 2, 2, BBLK], DT8, tag="xb")
                nc.gpsimd.dma_start(t, xb[:, blk])
                return t

            # NOTE: replicating the bias with gpsimd.partition_broadcast was
            # ~2.6us faster in the cost model but produced nondeterministic
            # wrong results on HW (stale partitions); the broadcast DMA is
            # the reliable path. It goes FIRST on SP so its write burst lands
            # in the initial PE-idle window instead of starving the
            # pass-major matmuls.
            c2b = cres.tile([P, K], mybir.dt.float32)
            nc.sync.dma_start(c2b, c2h[None, :].to_broadcast([P, K]))
            cw_sb = []
            for j in range(NJ):
                t = cres.tile([P, 2, 2, K], DT8, tag=f"c{j}")
                nc.sync.dma_start(t, cw[j])
                cw_sb.append(t)

            # block 0 arrives as 2 chunks so early passes start sooner
            blk0 = xp.tile([P, NJ, 2, 2, BBLK], DT8, tag="xb")
            for s in range(2):
                nc.gpsimd.dma_start(blk0[:, 2 * s:2 * s + 2], xb[:, 0, 2 * s:2 * s + 2])

            def mm(ps, x_t, i, p_, presub):
                j, tix = divmod(p_, 3)
                xsel, csel = TERMS[tix]
                first = (p_ == 0) and presub
                last = p_ == NPASS - 1
                w = x_t[:, j, :, xsel, i * P:(i + 1) * P]
                r = cw_sb[j][:, :, csel, :]
                nc.tensor.matmul(ps[:, 0:NH], w, r[:, :, 0:NH],
                                 start=first, stop=last,
                                 perf_mode=mybir.MatmulPerfMode.DoubleRow,
                                 skip_group_check=True)
                nc.tensor.matmul(ps[:, NH:K], w, r[:, :, NH:K],
                                 start=first, stop=last,
                                 perf_mode=mybir.MatmulPerfMode.DoubleRow,
                                 skip_group_check=True)

            def finish(src, t):
                mx = work.tile([P, 8], mybir.dt.float32, tag="mx")
                nc.vector.max(out=mx, in_=src)
                ix = work.tile([P, 8], mybir.dt.uint32, tag="ix")
                nc.vector.max_index(ix, mx, src)
                nc.sync.dma_start(out[t], ix[:, 0:1])
                nc.sync.dma_start(mxo[t], mx[:, 0:2])

            def reg_tile(x_t, t, nocopy):
                ps = psp.tile([P, K], mybir.dt.float32, tag="ps")
                nc.scalar.copy(ps, c2b)
                for p_ in range(NPASS):
                    mm(ps, x_t, t % TPB, p_, False)
                if nocopy:
                    # tail tiles: banks need no recycling, skip the
                    # copy-out hop to shorten the final serial chain
                    finish(ps, t)
                else:
                    a_sb = work.tile([P, K], mybir.dt.float32, tag="a")
                    nc.scalar.copy(a_sb, ps)
                    finish(a_sb, t)

            # block 0: first 3 tiles pass-major, bias on DVE, adds emitted
            # before maxes. Tile 3 takes the regular Act path (its PSUM bank
            # is never touched by the pass-major group), which warms the Act
            # copy-in/copy-out pipeline before block 1 needs it.
            PM = 3
            pss = []
            for i in range(PM):
                pstile = psp.tile([P, K], mybir.dt.float32, tag="ps")
                pss.append(pstile)
            for p_ in range(NPASS):
                for i in range(PM):
                    mm(pss[i], blk0, i, p_, True)
            srcs = []
            for i in range(PM):
                a_sb = work.tile([P, K], mybir.dt.float32, tag="a")
                nc.vector.tensor_add(a_sb, pss[i], c2b)
                srcs.append(a_sb)
            for i in range(PM):
                finish(srcs[i], i)
            for i in range(PM, TPB):
                reg_tile(blk0, i, False)

            for blk in range(1, nblk):
                x_t = load_blk(blk)
                for i in range(TPB):
                    t = blk * TPB + i
                    reg_tile(x_t, t, t >= ntiles - 4)
    nc.finalize()
    return nc


def _get_nc(bl):
    if bl not in _NC_CACHE:
        _NC_CACHE[bl] = _build(bl)
    return _NC_CACHE[bl]


def _make_in_maps(X, C):
    """Host prep: e4m3 plane splits + deduped DoubleRow layouts."""
    X0 = X.astype(E4)
    X1 = (X - X0.astype(np.float32)).astype(E4)
    C0 = C.astype(E4)
    C1 = (C - C0.astype(np.float32)).astype(E4)
    xsrcs = (np.ascontiguousarray(X0.T), np.ascontiguousarray(X1.T))
    csrcs = (np.ascontiguousarray(C0.T), np.ascontiguousarray(C1.T))

    cwt = np.empty((NJ, P, 2, 2, K), dtype=E4)
    for j in range(NJ):
        for i in range(2):
            f = 2 * j + i
            for s in range(2):
                cwt[j, :, i, s, :] = csrcs[s][f * P:(f + 1) * P, :]

    c2 = np.einsum("kf,kf->k", C.astype(np.float64), C.astype(np.float64))
    nc2h = (-0.5 * c2).astype(np.float32)

    in_maps = []
    for c in range(NCORES):
        b0 = c * BL
        xbc = np.empty((P, NBLK, NJ, 2, 2, BBLK), dtype=E4)
        for j in range(NJ):
            for i in range(2):
                f = 2 * j + i
                for s in range(2):
                    blkview = xsrcs[s][f * P:(f + 1) * P,
                                       b0:b0 + BL].reshape(P, NBLK, BBLK)
                    xbc[:, :, j, i, s, :] = blkview
        in_maps.append({"xb": xbc, "cw": cwt, "c2h": nc2h})
    return in_maps, c2


def kernel(X, centroids):
    X = np.ascontiguousarray(np.asarray(X, dtype=np.float32))
    C = np.ascontiguousarray(np.asarray(centroids, dtype=np.float32))
    assert X.shape == (B, F) and C.shape == (K, F)

    in_maps, c2 = _make_in_maps(X, C)
    nc = _get_nc(BL)

    # The device occasionally returns a corrupted run (observed ~1% of rows
    # with wildly wrong scores). Verify a 512-row sample of the device's
    # top-1 scores against exact host values (tolerance >> fp8 noise) and
    # relaunch on mismatch.
    rows = np.arange(0, B, B // 512)
    for _attempt in range(4):
        res = run_bass_kernel_spmd(nc, in_maps, core_ids=list(range(NCORES)))
        out = np.concatenate(
            [r["out"].reshape(-1) for r in res.results]).astype(np.int32)
        mx = np.concatenate([r["mxo"].reshape(-1, 2) for r in res.results])
        sc = np.einsum("rf,rf->r", X[rows].astype(np.float64),
                       C[out[rows]].astype(np.float64)) - 0.5 * c2[out[rows]]
        if np.max(np.abs(sc - mx[rows, 0])) < 1.0:
            break

    # exact host re-score of rows whose device top-2 gap is inside the fp8
    # noise margin: every fp8-induced argmin flip sits well below MARGIN
    gap = mx[:, 0] - mx[:, 1]
    risky = np.flatnonzero(gap < MARGIN)
    if risky.size:
        Xr = X[risky].astype(np.float64)
        d2 = (-2.0 * (Xr @ C.T.astype(np.float64))) + c2[None, :]
        out[risky] = np.argmin(d2, axis=1).astype(np.int32)
    return out


# revision 16
# speedup vs baseline: 1.0089x; 1.0005x over previous
"""KMeans assignment (vq_codebook) Trainium2 kernel.

argmin_k ||x_b - c_k||^2 for X[65536,1024], C[1024,1024], 8 NeuronCores,
data-parallel over the batch (8192 rows/core), centroids replicated.

Math: argmin_k d2 = argmax_k (X@C^T - ||c||^2/2); row term ||x||^2 dropped.

The cross term runs entirely on the PE in fp8 (e4m3) DoubleRow perf mode at
0.5 cycles/row — 2x the bf16/fp32r rate. Operands are split into two e4m3
planes each (X = X0+X1, C = C0+C1) and the three dominant product terms
  X0@C0 + X0@C1 + X1@C0
are computed with DoubleRow packing two (weight,ifmap) plane pairs per
matmul: 24 plane-terms/tile -> 24 instructions x 512 cols x 0.5 cycles
= 6144 PE cycles/tile, vs 8192 for a single fp32r pass and 24576 for the
original hi/lo bf16 3-pass scheme. The shared X0/C0 planes are stored once
and re-read via strided plane APs (33% less X DMA traffic).

The -||c||^2/2 bias (host fp64) is preloaded into each PSUM tile by the
otherwise-idle Activation engine (matmuls accumulate on top, start=False);
finished score tiles are copied PSUM->SBUF by the Activation engine so the
PSUM banks recycle ~2.3us earlier and the DVE (max + max_index per tile)
skips the PSUM access penalty — the DVE runs at ~91% of the PE pace, so
this slack is what keeps the PE from stalling. The last 4 tiles skip the
copy-out (no recycling pressure) to shorten the final serial chain.

Block 0 runs pass-major across its 4 PSUM-resident tiles (PE consumption
per pass ~= the C-chunk DMA arrival pace, so the PE rides the C stream
instead of idling) and biases on the DVE (start=True); its 4 bias-adds are
emitted before the max scans so the PSUM banks release at ~1.2us intervals
instead of ~3.5us.

Accuracy: the dropped X1@C1 term and e4m3 quantization give the device
scores a d2 noise std of ~0.04, flipping ~257 of 65536 argmins. Every
device tile also ships its top-2 score values (free: DVE max already
computes them); the host exactly re-scores rows whose top-2 gap is inside
MARGIN=0.25 (~7.5% of rows; every observed flip on HW sits below gap
0.084, a 3x margin). Device computes 100% of the B*K scores and argmaxes;
the host re-check makes the result exact to fp64 for all flagged rows.
"""
import numpy as np
import ml_dtypes
import concourse.bacc as bacc
import concourse.mybir as mybir
from concourse.tile import TileContext
from concourse.bass_utils import run_bass_kernel_spmd

B, F, K = 65536, 1024, 1024
NCORES = 8
BL = B // NCORES          # rows per core
P = 128
FCH = F // P              # 8 feature chunks
NJ = FCH // 2             # 4 chunk pairs
NPASS = 3 * NJ            # 12 DoubleRow passes per tile-half
NH = 512                  # psum half (max fp32 moving operand / bank)
BBLK = 1024               # rows per X DMA block
NBLK = BL // BBLK
TPB = BBLK // P           # b-tiles per block
MARGIN = 0.25             # host re-score threshold on the top-2 score gap
E4 = ml_dtypes.float8_e4m3
DT8 = mybir.dt.float8e4

# pass 3j+t covers feature chunks (2j, 2j+1) with term t: (x_plane, c_plane)
TERMS = [(0, 0), (0, 1), (1, 0)]   # X0@C0, X0@C1, X1@C0

_NC_CACHE = {}


def _build(bl):
    nblk = bl // BBLK
    nb = bl // P
    ntiles = nblk * TPB
    nc = bacc.Bacc("TRN2", target_bir_lowering=False)
    # deduped planes: X [P, blk, j, i(chunk-in-pair), s(X0|X1), BBLK]
    xb = nc.dram_tensor("xb", [P, nblk, NJ, 2, 2, BBLK], DT8, kind="ExternalInput")
    # C [j, P, i, s(C0|C1), K]
    cw = nc.dram_tensor("cw", [NJ, P, 2, 2, K], DT8, kind="ExternalInput")
    c2h = nc.dram_tensor("c2h", [K], mybir.dt.float32, kind="ExternalInput")
    out = nc.dram_tensor("out", [nb, P, 1], mybir.dt.uint32, kind="ExternalOutput")
    mxo = nc.dram_tensor("mxo", [nb, P, 2], mybir.dt.float32, kind="ExternalOutput")

    with TileContext(nc) as tc:
        with (
            tc.tile_pool(name="cres", bufs=1) as cres,
            tc.tile_pool(name="xp", bufs=2) as xp,
            tc.tile_pool(name="work", bufs=6) as work,
            tc.tile_pool(name="psp", bufs=4, space="PSUM") as psp,
        ):
            # X blocks on the Pool queue (one contiguous DMA per block),
            # C/c2/outputs on SP, bias copy-in + score copy-out on Act.
            def load_blk(blk):
                t = xp.tile([P, NJ, 2, 2, BBLK], DT8, tag="xb")
                nc.gpsimd.dma_start(t, xb[:, blk])
                return t

            # NOTE: replicating the bias with gpsimd.partition_broadcast was
            # ~2.6us faster in the cost model but produced nondeterministic
            # wrong results on HW (stale partitions); the broadcast DMA is
            # the reliable path. It goes FIRST on SP so its write burst lands
            # in the initial PE-idle window instead of starving the
            # pass-major matmuls.
            c2b = cres.tile([P, K], mybir.dt.float32)
            nc.sync.dma_start(c2b, c2h[None, :].to_broadcast([P, K]))
            cw_sb = []
            for j in range(NJ):
                t = cres.tile([P, 2, 2, K], DT8, tag=f"c{j}")
                nc.sync.dma_start(t, cw[j])
                cw_sb.append(t)

            # block 0 arrives as 4 chunks so early passes start sooner
            blk0 = xp.tile([P, NJ, 2, 2, BBLK], DT8, tag="xb")
            for s in range(NJ):
                nc.gpsimd.dma_start(blk0[:, s:s + 1], xb[:, 0, s:s + 1])

            def mm(ps, x_t, i, p_, presub):
                j, tix = divmod(p_, 3)
                xsel, csel = TERMS[tix]
                first = (p_ == 0) and presub
                last = p_ == NPASS - 1
                w = x_t[:, j, :, xsel, i * P:(i + 1) * P]
                r = cw_sb[j][:, :, csel, :]
                nc.tensor.matmul(ps[:, 0:NH], w, r[:, :, 0:NH],
                                 start=first, stop=last,
                                 perf_mode=mybir.MatmulPerfMode.DoubleRow,
                                 skip_group_check=True)
                nc.tensor.matmul(ps[:, NH:K], w, r[:, :, NH:K],
                                 start=first, stop=last,
                                 perf_mode=mybir.MatmulPerfMode.DoubleRow,
                                 skip_group_check=True)

            def finish(src, t):
                mx = work.tile([P, 8], mybir.dt.float32, tag="mx")
                nc.vector.max(out=mx, in_=src)
                ix = work.tile([P, 8], mybir.dt.uint32, tag="ix")
                nc.vector.max_index(ix, mx, src)
                nc.sync.dma_start(out[t], ix[:, 0:1])
                nc.sync.dma_start(mxo[t], mx[:, 0:2])

            def reg_tile(x_t, t, nocopy):
                ps = psp.tile([P, K], mybir.dt.float32, tag="ps")
                nc.scalar.copy(ps, c2b)
                for p_ in range(NPASS):
                    mm(ps, x_t, t % TPB, p_, False)
                if nocopy:
                    # tail tiles: banks need no recycling, skip the
                    # copy-out hop to shorten the final serial chain
                    finish(ps, t)
                else:
                    a_sb = work.tile([P, K], mybir.dt.float32, tag="a")
                    nc.scalar.copy(a_sb, ps)
                    finish(a_sb, t)

            # block 0: first 3 tiles pass-major, bias on DVE, adds emitted
            # before maxes. Tile 3 takes the regular Act path (its PSUM bank
            # is never touched by the pass-major group), which warms the Act
            # copy-in/copy-out pipeline before block 1 needs it.
            PM = 3
            pss = []
            for i in range(PM):
                pstile = psp.tile([P, K], mybir.dt.float32, tag="ps")
                pss.append(pstile)
            for p_ in range(NPASS):
                for i in range(PM):
                    mm(pss[i], blk0, i, p_, True)
            srcs = []
            for i in range(PM):
                a_sb = work.tile([P, K], mybir.dt.float32, tag="a")
                nc.vector.tensor_add(a_sb, pss[i], c2b)
                srcs.append(a_sb)
            for i in range(PM):
                finish(srcs[i], i)
            for i in range(PM, TPB):
                reg_tile(blk0, i, False)

            for blk in range(1, nblk):
                x_t = load_blk(blk)
                for i in range(TPB):
                    t = blk * TPB + i
                    reg_tile(x_t, t, t >= ntiles - 8)
    nc.finalize()
    return nc


def _get_nc(bl):
    if bl not in _NC_CACHE:
        _NC_CACHE[bl] = _build(bl)
    return _NC_CACHE[bl]


def _make_in_maps(X, C):
    """Host prep: e4m3 plane splits + deduped DoubleRow layouts."""
    X0 = X.astype(E4)
    X1 = (X - X0.astype(np.float32)).astype(E4)
    C0 = C.astype(E4)
    C1 = (C - C0.astype(np.float32)).astype(E4)
    xsrcs = (np.ascontiguousarray(X0.T), np.ascontiguousarray(X1.T))
    csrcs = (np.ascontiguousarray(C0.T), np.ascontiguousarray(C1.T))

    cwt = np.empty((NJ, P, 2, 2, K), dtype=E4)
    for j in range(NJ):
        for i in range(2):
            f = 2 * j + i
            for s in range(2):
                cwt[j, :, i, s, :] = csrcs[s][f * P:(f + 1) * P, :]

    c2 = np.einsum("kf,kf->k", C.astype(np.float64), C.astype(np.float64))
    nc2h = (-0.5 * c2).astype(np.float32)

    in_maps = []
    for c in range(NCORES):
        b0 = c * BL
        xbc = np.empty((P, NBLK, NJ, 2, 2, BBLK), dtype=E4)
        for j in range(NJ):
            for i in range(2):
                f = 2 * j + i
                for s in range(2):
                    blkview = xsrcs[s][f * P:(f + 1) * P,
                                       b0:b0 + BL].reshape(P, NBLK, BBLK)
                    xbc[:, :, j, i, s, :] = blkview
        in_maps.append({"xb": xbc, "cw": cwt, "c2h": nc2h})
    return in_maps, c2


def kernel(X, centroids):
    X = np.ascontiguousarray(np.asarray(X, dtype=np.float32))
    C = np.ascontiguousarray(np.asarray(centroids, dtype=np.float32))
    assert X.shape == (B, F) and C.shape == (K, F)

    in_maps, c2 = _make_in_maps(X, C)
    nc = _get_nc(BL)

    # The device occasionally returns a corrupted run (observed ~1% of rows
    # with wildly wrong scores). Verify a 512-row sample of the device's
    # top-1 scores against exact host values (tolerance >> fp8 noise) and
    # relaunch on mismatch.
    rows = np.arange(0, B, B // 512)
    for _attempt in range(4):
        res = run_bass_kernel_spmd(nc, in_maps, core_ids=list(range(NCORES)))
        out = np.concatenate(
            [r["out"].reshape(-1) for r in res.results]).astype(np.int32)
        mx = np.concatenate([r["mxo"].reshape(-1, 2) for r in res.results])
        sc = np.einsum("rf,rf->r", X[rows].astype(np.float64),
                       C[out[rows]].astype(np.float64)) - 0.5 * c2[out[rows]]
        if np.max(np.abs(sc - mx[rows, 0])) < 1.0:
            break

    # exact host re-score of rows whose device top-2 gap is inside the fp8
    # noise margin: every fp8-induced argmin flip sits well below MARGIN
    gap = mx[:, 0] - mx[:, 1]
    risky = np.flatnonzero(gap < MARGIN)
    if risky.size:
        Xr = X[risky].astype(np.float64)
        d2 = (-2.0 * (Xr @ C.T.astype(np.float64))) + c2[None, :]
        out[risky] = np.argmin(d2, axis=1).astype(np.int32)
    return out


# revision 17
# speedup vs baseline: 1.0122x; 1.0032x over previous
"""KMeans assignment (vq_codebook) Trainium2 kernel.

argmin_k ||x_b - c_k||^2 for X[65536,1024], C[1024,1024], 8 NeuronCores,
data-parallel over the batch (8192 rows/core), centroids replicated.

Math: argmin_k d2 = argmax_k (X@C^T - ||c||^2/2); row term ||x||^2 dropped.

The cross term runs entirely on the PE in fp8 (e4m3) DoubleRow perf mode at
0.5 cycles/row — 2x the bf16/fp32r rate. Operands are split into two e4m3
planes each (X = X0+X1, C = C0+C1) and the three dominant product terms
  X0@C0 + X0@C1 + X1@C0
are computed with DoubleRow packing two (weight,ifmap) plane pairs per
matmul: 24 plane-terms/tile -> 24 instructions x 512 cols x 0.5 cycles
= 6144 PE cycles/tile, vs 8192 for a single fp32r pass and 24576 for the
original hi/lo bf16 3-pass scheme. The shared X0/C0 planes are stored once
and re-read via strided plane APs (33% less X DMA traffic).

The -||c||^2/2 bias (host fp64) is preloaded into each PSUM tile by the
otherwise-idle Activation engine (matmuls accumulate on top, start=False);
finished score tiles are copied PSUM->SBUF by the Activation engine so the
PSUM banks recycle ~2.3us earlier and the DVE (max + max_index per tile)
skips the PSUM access penalty — the DVE runs at ~91% of the PE pace, so
this slack is what keeps the PE from stalling. The last 4 tiles skip the
copy-out (no recycling pressure) to shorten the final serial chain.

Block 0 runs pass-major across its 4 PSUM-resident tiles (PE consumption
per pass ~= the C-chunk DMA arrival pace, so the PE rides the C stream
instead of idling) and biases on the DVE (start=True); its 4 bias-adds are
emitted before the max scans so the PSUM banks release at ~1.2us intervals
instead of ~3.5us.

Accuracy: the dropped X1@C1 term and e4m3 quantization give the device
scores a d2 noise std of ~0.04, flipping ~257 of 65536 argmins. Every
device tile also ships its top-2 score values (free: DVE max already
computes them); the host exactly re-scores rows whose top-2 gap is inside
MARGIN=0.25 (~7.5% of rows; every observed flip on HW sits below gap
0.084, a 3x margin). Device computes 100% of the B*K scores and argmaxes;
the host re-check makes the result exact to fp64 for all flagged rows.
"""
import numpy as np
import ml_dtypes
import concourse.bacc as bacc
import concourse.mybir as mybir
from concourse.tile import TileContext
from concourse.bass_utils import run_bass_kernel_spmd

B, F, K = 65536, 1024, 1024
NCORES = 8
BL = B // NCORES          # rows per core
P = 128
FCH = F // P              # 8 feature chunks
NJ = FCH // 2             # 4 chunk pairs
NPASS = 3 * NJ            # 12 DoubleRow passes per tile-half
NH = 512                  # psum half (max fp32 moving operand / bank)
BBLK = 1024               # rows per X DMA block
NBLK = BL // BBLK
TPB = BBLK // P           # b-tiles per block
MARGIN = 0.25             # host re-score threshold on the top-2 score gap
E4 = ml_dtypes.float8_e4m3
DT8 = mybir.dt.float8e4

# pass 3j+t covers feature chunks (2j, 2j+1) with term t: (x_plane, c_plane)
TERMS = [(0, 0), (0, 1), (1, 0)]   # X0@C0, X0@C1, X1@C0

_NC_CACHE = {}


def _build(bl):
    nblk = bl // BBLK
    nb = bl // P
    ntiles = nblk * TPB
    nc = bacc.Bacc("TRN2", target_bir_lowering=False)
    # deduped planes: X [P, blk, j, i(chunk-in-pair), s(X0|X1), BBLK]
    xb = nc.dram_tensor("xb", [P, nblk, NJ, 2, 2, BBLK], DT8, kind="ExternalInput")
    # C [j, P, i, s(C0|C1), K]
    cw = nc.dram_tensor("cw", [NJ, P, 2, 2, K], DT8, kind="ExternalInput")
    c2h = nc.dram_tensor("c2h", [K], mybir.dt.float32, kind="ExternalInput")
    out = nc.dram_tensor("out", [nb, P, 1], mybir.dt.uint32, kind="ExternalOutput")
    mxo = nc.dram_tensor("mxo", [nb, P, 2], mybir.dt.float32, kind="ExternalOutput")

    with TileContext(nc) as tc:
        with (
            tc.tile_pool(name="cres", bufs=1) as cres,
            tc.tile_pool(name="xp", bufs=2) as xp,
            tc.tile_pool(name="work", bufs=6) as work,
            tc.tile_pool(name="psp", bufs=4, space="PSUM") as psp,
        ):
            # X blocks on the Pool queue (one contiguous DMA per block),
            # C/c2/outputs on SP, bias copy-in + score copy-out on Act.
            def load_blk(blk):
                t = xp.tile([P, NJ, 2, 2, BBLK], DT8, tag="xb")
                nc.gpsimd.dma_start(t, xb[:, blk])
                return t

            # NOTE: replicating the bias with gpsimd.partition_broadcast was
            # ~2.6us faster in the cost model but produced nondeterministic
            # wrong results on HW (stale partitions); the broadcast DMA is
            # the reliable path. It goes FIRST on SP so its write burst lands
            # in the initial PE-idle window instead of starving the
            # pass-major matmuls.
            c2b = cres.tile([P, K], mybir.dt.float32)
            nc.sync.dma_start(c2b, c2h[None, :].to_broadcast([P, K]))
            cw_sb = []
            for j in range(NJ):
                t = cres.tile([P, 2, 2, K], DT8, tag=f"c{j}")
                nc.sync.dma_start(t, cw[j])
                cw_sb.append(t)

            # block 0 arrives as 4 chunks so early passes start sooner
            blk0 = xp.tile([P, NJ, 2, 2, BBLK], DT8, tag="xb")
            for s in range(NJ):
                nc.gpsimd.dma_start(blk0[:, s:s + 1], xb[:, 0, s:s + 1])

            def mm(ps, x_t, i, p_, presub):
                j, tix = divmod(p_, 3)
                xsel, csel = TERMS[tix]
                first = (p_ == 0) and presub
                last = p_ == NPASS - 1
                w = x_t[:, j, :, xsel, i * P:(i + 1) * P]
                r = cw_sb[j][:, :, csel, :]
                nc.tensor.matmul(ps[:, 0:NH], w, r[:, :, 0:NH],
                                 start=first, stop=last,
                                 perf_mode=mybir.MatmulPerfMode.DoubleRow,
                                 skip_group_check=True)
                nc.tensor.matmul(ps[:, NH:K], w, r[:, :, NH:K],
                                 start=first, stop=last,
                                 perf_mode=mybir.MatmulPerfMode.DoubleRow,
                                 skip_group_check=True)

            def finish(src, t):
                mx = work.tile([P, 8], mybir.dt.float32, tag="mx")
                nc.vector.max(out=mx, in_=src)
                ix = work.tile([P, 8], mybir.dt.uint32, tag="ix")
                nc.vector.max_index(ix, mx, src)
                nc.sync.dma_start(out[t], ix[:, 0:1])
                nc.sync.dma_start(mxo[t], mx[:, 0:2])

            def reg_tile(x_t, t, nocopy):
                ps = psp.tile([P, K], mybir.dt.float32, tag="ps")
                nc.scalar.copy(ps, c2b)
                for p_ in range(NPASS):
                    mm(ps, x_t, t % TPB, p_, False)
                if nocopy:
                    # tail tiles: banks need no recycling, skip the
                    # copy-out hop to shorten the final serial chain
                    finish(ps, t)
                else:
                    a_sb = work.tile([P, K], mybir.dt.float32, tag="a")
                    nc.scalar.copy(a_sb, ps)
                    finish(a_sb, t)

            # block 0: first 3 tiles pass-major, bias on DVE, adds emitted
            # before maxes. Tile 3 takes the regular Act path (its PSUM bank
            # is never touched by the pass-major group), which warms the Act
            # copy-in/copy-out pipeline before block 1 needs it.
            PM = 3
            pss = []
            for i in range(PM):
                pstile = psp.tile([P, K], mybir.dt.float32, tag="ps")
                pss.append(pstile)
            for p_ in range(NPASS):
                for i in range(PM):
                    mm(pss[i], blk0, i, p_, True)
            srcs = []
            for i in range(PM):
                a_sb = work.tile([P, K], mybir.dt.float32, tag="a")
                nc.vector.tensor_add(a_sb, pss[i], c2b)
                srcs.append(a_sb)
            for i in range(PM):
                finish(srcs[i], i)
            for i in range(PM, TPB):
                reg_tile(blk0, i, False)

            for blk in range(1, nblk):
                x_t = load_blk(blk)
                for i in range(TPB):
                    t = blk * TPB + i
                    reg_tile(x_t, t, t >= ntiles - 32)
    nc.finalize()
    return nc


def _get_nc(bl):
    if bl not in _NC_CACHE:
        _NC_CACHE[bl] = _build(bl)
    return _NC_CACHE[bl]


def _make_in_maps(X, C):
    """Host prep: e4m3 plane splits + deduped DoubleRow layouts."""
    X0 = X.astype(E4)
    X1 = (X - X0.astype(np.float32)).astype(E4)
    C0 = C.astype(E4)
    C1 = (C - C0.astype(np.float32)).astype(E4)
    xsrcs = (np.ascontiguousarray(X0.T), np.ascontiguousarray(X1.T))
    csrcs = (np.ascontiguousarray(C0.T), np.ascontiguousarray(C1.T))

    cwt = np.empty((NJ, P, 2, 2, K), dtype=E4)
    for j in range(NJ):
        for i in range(2):
            f = 2 * j + i
            for s in range(2):
                cwt[j, :, i, s, :] = csrcs[s][f * P:(f + 1) * P, :]

    c2 = np.einsum("kf,kf->k", C.astype(np.float64), C.astype(np.float64))
    nc2h = (-0.5 * c2).astype(np.float32)

    in_maps = []
    for c in range(NCORES):
        b0 = c * BL
        xbc = np.empty((P, NBLK, NJ, 2, 2, BBLK), dtype=E4)
        for j in range(NJ):
            for i in range(2):
                f = 2 * j + i
                for s in range(2):
                    blkview = xsrcs[s][f * P:(f + 1) * P,
                                       b0:b0 + BL].reshape(P, NBLK, BBLK)
                    xbc[:, :, j, i, s, :] = blkview
        in_maps.append({"xb": xbc, "cw": cwt, "c2h": nc2h})
    return in_maps, c2


def kernel(X, centroids):
    X = np.ascontiguousarray(np.asarray(X, dtype=np.float32))
    C = np.ascontiguousarray(np.asarray(centroids, dtype=np.float32))
    assert X.shape == (B, F) and C.shape == (K, F)

    in_maps, c2 = _make_in_maps(X, C)
    nc = _get_nc(BL)

    # The device occasionally returns a corrupted run (observed ~1% of rows
    # with wildly wrong scores). Verify a 512-row sample of the device's
    # top-1 scores against exact host values (tolerance >> fp8 noise) and
    # relaunch on mismatch.
    rows = np.arange(0, B, B // 512)
    for _attempt in range(4):
        res = run_bass_kernel_spmd(nc, in_maps, core_ids=list(range(NCORES)))
        out = np.concatenate(
            [r["out"].reshape(-1) for r in res.results]).astype(np.int32)
        mx = np.concatenate([r["mxo"].reshape(-1, 2) for r in res.results])
        sc = np.einsum("rf,rf->r", X[rows].astype(np.float64),
                       C[out[rows]].astype(np.float64)) - 0.5 * c2[out[rows]]
        if np.max(np.abs(sc - mx[rows, 0])) < 1.0:
            break

    # exact host re-score of rows whose device top-2 gap is inside the fp8
    # noise margin: every fp8-induced argmin flip sits well below MARGIN
    gap = mx[:, 0] - mx[:, 1]
    risky = np.flatnonzero(gap < MARGIN)
    if risky.size:
        Xr = X[risky].astype(np.float64)
        d2 = (-2.0 * (Xr @ C.T.astype(np.float64))) + c2[None, :]
        out[risky] = np.argmin(d2, axis=1).astype(np.int32)
    return out


# revision 18
# speedup vs baseline: 1.0133x; 1.0011x over previous
"""KMeans assignment (vq_codebook) Trainium2 kernel.

argmin_k ||x_b - c_k||^2 for X[65536,1024], C[1024,1024], 8 NeuronCores,
data-parallel over the batch (8192 rows/core), centroids replicated.

Math: argmin_k d2 = argmax_k (X@C^T - ||c||^2/2); row term ||x||^2 dropped.

The cross term runs entirely on the PE in fp8 (e4m3) DoubleRow perf mode at
0.5 cycles/row — 2x the bf16/fp32r rate. Operands are split into two e4m3
planes each (X = X0+X1, C = C0+C1) and the three dominant product terms
  X0@C0 + X0@C1 + X1@C0
are computed with DoubleRow packing two (weight,ifmap) plane pairs per
matmul: 24 plane-terms/tile -> 24 instructions x 512 cols x 0.5 cycles
= 6144 PE cycles/tile, vs 8192 for a single fp32r pass and 24576 for the
original hi/lo bf16 3-pass scheme. The shared X0/C0 planes are stored once
and re-read via strided plane APs (33% less X DMA traffic).

The -||c||^2/2 bias (host fp64) is preloaded into each PSUM tile by the
otherwise-idle Activation engine (matmuls accumulate on top, start=False);
finished score tiles are copied PSUM->SBUF by the Activation engine so the
PSUM banks recycle ~2.3us earlier and the DVE (max + max_index per tile)
skips the PSUM access penalty — the DVE runs at ~91% of the PE pace, so
this slack is what keeps the PE from stalling. The last 4 tiles skip the
copy-out (no recycling pressure) to shorten the final serial chain.

Block 0 runs pass-major across its 4 PSUM-resident tiles (PE consumption
per pass ~= the C-chunk DMA arrival pace, so the PE rides the C stream
instead of idling) and biases on the DVE (start=True); its 4 bias-adds are
emitted before the max scans so the PSUM banks release at ~1.2us intervals
instead of ~3.5us.

Accuracy: the dropped X1@C1 term and e4m3 quantization give the device
scores a d2 noise std of ~0.04, flipping ~257 of 65536 argmins. Every
device tile also ships its top-2 score values (free: DVE max already
computes them); the host exactly re-scores rows whose top-2 gap is inside
MARGIN=0.25 (~7.5% of rows; every observed flip on HW sits below gap
0.084, a 3x margin). Device computes 100% of the B*K scores and argmaxes;
the host re-check makes the result exact to fp64 for all flagged rows.
"""
import numpy as np
import ml_dtypes
import concourse.bacc as bacc
import concourse.mybir as mybir
from concourse.tile import TileContext
from concourse.bass_utils import run_bass_kernel_spmd

B, F, K = 65536, 1024, 1024
NCORES = 8
BL = B // NCORES          # rows per core
P = 128
FCH = F // P              # 8 feature chunks
NJ = FCH // 2             # 4 chunk pairs
NPASS = 3 * NJ            # 12 DoubleRow passes per tile-half
NH = 512                  # psum half (max fp32 moving operand / bank)
BBLK = 1024               # rows per X DMA block
NBLK = BL // BBLK
TPB = BBLK // P           # b-tiles per block
MARGIN = 0.25             # host re-score threshold on the top-2 score gap
E4 = ml_dtypes.float8_e4m3
DT8 = mybir.dt.float8e4

# pass 3j+t covers feature chunks (2j, 2j+1) with term t: (x_plane, c_plane)
TERMS = [(0, 0), (0, 1), (1, 0)]   # X0@C0, X0@C1, X1@C0

_NC_CACHE = {}


def _build(bl):
    nblk = bl // BBLK
    nb = bl // P
    ntiles = nblk * TPB
    nc = bacc.Bacc("TRN2", target_bir_lowering=False)
    # deduped planes: X [P, blk, j, i(chunk-in-pair), s(X0|X1), BBLK]
    xb = nc.dram_tensor("xb", [P, nblk, NJ, 2, 2, BBLK], DT8, kind="ExternalInput")
    # C [j, P, i, s(C0|C1), K]
    cw = nc.dram_tensor("cw", [NJ, P, 2, 2, K], DT8, kind="ExternalInput")
    c2h = nc.dram_tensor("c2h", [K], mybir.dt.float32, kind="ExternalInput")
    out = nc.dram_tensor("out", [nb, P, 1], mybir.dt.uint32, kind="ExternalOutput")
    mxo = nc.dram_tensor("mxo", [nb, P, 2], mybir.dt.float32, kind="ExternalOutput")

    with TileContext(nc) as tc:
        with (
            tc.tile_pool(name="cres", bufs=1) as cres,
            tc.tile_pool(name="xp", bufs=2) as xp,
            tc.tile_pool(name="work", bufs=6) as work,
            tc.tile_pool(name="psp", bufs=4, space="PSUM") as psp,
        ):
            # X blocks on the Pool queue (one contiguous DMA per block),
            # C/c2/outputs on SP, bias copy-in + score copy-out on Act.
            def load_blk(blk):
                t = xp.tile([P, NJ, 2, 2, BBLK], DT8, tag="xb")
                nc.gpsimd.dma_start(t, xb[:, blk])
                return t

            # NOTE: replicating the bias with gpsimd.partition_broadcast was
            # ~2.6us faster in the cost model but produced nondeterministic
            # wrong results on HW (stale partitions); the broadcast DMA is
            # the reliable path. It goes FIRST on SP so its write burst lands
            # in the initial PE-idle window instead of starving the
            # pass-major matmuls.
            c2b = cres.tile([P, K], mybir.dt.float32)
            nc.sync.dma_start(c2b, c2h[None, :].to_broadcast([P, K]))
            cw_sb = []
            for j in range(NJ):
                t = cres.tile([P, 2, 2, K], DT8, tag=f"c{j}")
                nc.sync.dma_start(t, cw[j])
                cw_sb.append(t)

            # block 0 arrives as 4 chunks so early passes start sooner
            blk0 = xp.tile([P, NJ, 2, 2, BBLK], DT8, tag="xb")
            for s in range(NJ):
                nc.gpsimd.dma_start(blk0[:, s:s + 1], xb[:, 0, s:s + 1])

            def mm(ps, x_t, i, p_, presub):
                j, tix = divmod(p_, 3)
                xsel, csel = TERMS[tix]
                first = (p_ == 0) and presub
                last = p_ == NPASS - 1
                w = x_t[:, j, :, xsel, i * P:(i + 1) * P]
                r = cw_sb[j][:, :, csel, :]
                nc.tensor.matmul(ps[:, 0:NH], w, r[:, :, 0:NH],
                                 start=first, stop=last,
                                 perf_mode=mybir.MatmulPerfMode.DoubleRow,
                                 skip_group_check=True)
                nc.tensor.matmul(ps[:, NH:K], w, r[:, :, NH:K],
                                 start=first, stop=last,
                                 perf_mode=mybir.MatmulPerfMode.DoubleRow,
                                 skip_group_check=True)

            def finish(src, t):
                mx = work.tile([P, 8], mybir.dt.float32, tag="mx")
                nc.vector.max(out=mx, in_=src)
                ix = work.tile([P, 8], mybir.dt.uint32, tag="ix")
                nc.vector.max_index(ix, mx, src)
                nc.sync.dma_start(out[t], ix[:, 0:1])
                nc.sync.dma_start(mxo[t], mx[:, 0:2])

            def reg_tile(x_t, t, nocopy):
                ps = psp.tile([P, K], mybir.dt.float32, tag="ps")
                nc.scalar.copy(ps, c2b)
                for p_ in range(NPASS):
                    mm(ps, x_t, t % TPB, p_, False)
                if nocopy:
                    # tail tiles: banks need no recycling, skip the
                    # copy-out hop to shorten the final serial chain
                    finish(ps, t)
                else:
                    a_sb = work.tile([P, K], mybir.dt.float32, tag="a")
                    nc.scalar.copy(a_sb, ps)
                    finish(a_sb, t)

            # block 0: first 3 tiles pass-major, bias on DVE, adds emitted
            # before maxes. Tile 3 takes the regular Act path (its PSUM bank
            # is never touched by the pass-major group), which warms the Act
            # copy-in/copy-out pipeline before block 1 needs it.
            PM = 3
            pss = []
            for i in range(PM):
                pstile = psp.tile([P, K], mybir.dt.float32, tag="ps")
                pss.append(pstile)
            for p_ in range(NPASS):
                for i in range(PM):
                    mm(pss[i], blk0, i, p_, True)
            srcs = []
            for i in range(PM):
                a_sb = work.tile([P, K], mybir.dt.float32, tag="a")
                nc.vector.tensor_add(a_sb, pss[i], c2b)
                srcs.append(a_sb)
            for i in range(PM):
                finish(srcs[i], i)
            for i in range(PM, TPB):
                reg_tile(blk0, i, False)

            for blk in range(1, nblk):
                x_t = load_blk(blk)
                for i in range(TPB):
                    t = blk * TPB + i
                    reg_tile(x_t, t, t >= ntiles - 40)
    nc.finalize()
    return nc


def _get_nc(bl):
    if bl not in _NC_CACHE:
        _NC_CACHE[bl] = _build(bl)
    return _NC_CACHE[bl]


def _make_in_maps(X, C):
    """Host prep: e4m3 plane splits + deduped DoubleRow layouts."""
    X0 = X.astype(E4)
    X1 = (X - X0.astype(np.float32)).astype(E4)
    C0 = C.astype(E4)
    C1 = (C - C0.astype(np.float32)).astype(E4)
    xsrcs = (np.ascontiguousarray(X0.T), np.ascontiguousarray(X1.T))
    csrcs = (np.ascontiguousarray(C0.T), np.ascontiguousarray(C1.T))

    cwt = np.empty((NJ, P, 2, 2, K), dtype=E4)
    for j in range(NJ):
        for i in range(2):
            f = 2 * j + i
            for s in range(2):
                cwt[j, :, i, s, :] = csrcs[s][f * P:(f + 1) * P, :]

    c2 = np.einsum("kf,kf->k", C.astype(np.float64), C.astype(np.float64))
    nc2h = (-0.5 * c2).astype(np.float32)

    in_maps = []
    for c in range(NCORES):
        b0 = c * BL
        xbc = np.empty((P, NBLK, NJ, 2, 2, BBLK), dtype=E4)
        for j in range(NJ):
            for i in range(2):
                f = 2 * j + i
                for s in range(2):
                    blkview = xsrcs[s][f * P:(f + 1) * P,
                                       b0:b0 + BL].reshape(P, NBLK, BBLK)
                    xbc[:, :, j, i, s, :] = blkview
        in_maps.append({"xb": xbc, "cw": cwt, "c2h": nc2h})
    return in_maps, c2


def kernel(X, centroids):
    X = np.ascontiguousarray(np.asarray(X, dtype=np.float32))
    C = np.ascontiguousarray(np.asarray(centroids, dtype=np.float32))
    assert X.shape == (B, F) and C.shape == (K, F)

    in_maps, c2 = _make_in_maps(X, C)
    nc = _get_nc(BL)

    # The device occasionally returns a corrupted run (observed ~1% of rows
    # with wildly wrong scores). Verify a 512-row sample of the device's
    # top-1 scores against exact host values (tolerance >> fp8 noise) and
    # relaunch on mismatch.
    rows = np.arange(0, B, B // 512)
    for _attempt in range(4):
        res = run_bass_kernel_spmd(nc, in_maps, core_ids=list(range(NCORES)))
        out = np.concatenate(
            [r["out"].reshape(-1) for r in res.results]).astype(np.int32)
        mx = np.concatenate([r["mxo"].reshape(-1, 2) for r in res.results])
        sc = np.einsum("rf,rf->r", X[rows].astype(np.float64),
                       C[out[rows]].astype(np.float64)) - 0.5 * c2[out[rows]]
        if np.max(np.abs(sc - mx[rows, 0])) < 1.0:
            break

    # exact host re-score of rows whose device top-2 gap is inside the fp8
    # noise margin: every fp8-induced argmin flip sits well below MARGIN
    gap = mx[:, 0] - mx[:, 1]
    risky = np.flatnonzero(gap < MARGIN)
    if risky.size:
        Xr = X[risky].astype(np.float64)
        d2 = (-2.0 * (Xr @ C.T.astype(np.float64))) + c2[None, :]
        out[risky] = np.argmin(d2, axis=1).astype(np.int32)
    return out
